# revision 1
# baseline (speedup 1.0000x reference)
"""Trainium2 Bass kernel for a sparse-attention EncoderLayer.

Sharding: rows (L) are split into 8 contiguous shards of L/8; each edge is
owned by the core that owns its destination row (row_index is sorted, so each
core's edges are a contiguous range).  Each core computes Q/K/V for its row
shard, the K/V shards are AllGathered (bf16) so every core holds the full
K/V table in HBM, and per-edge K/V rows are fetched with dma_gather.  The
segment softmax is computed without the max-subtraction (scores here are
bounded by ~|q||k|/8 + |bias| < 10, so exp() cannot overflow in f32 and
alpha = exp(s - m)/sum exp(s - m) == exp(s)/sum exp(s)).  The alpha-weighted
scatter and the per-row softmax sums are evaluated as one-hot PE matmuls over
128-edge tiles, accumulated in PSUM per 128-row block.
"""

import math
import numpy as np
from contextlib import ExitStack

from ml_dtypes import bfloat16

import concourse.bass as bass
import concourse.mybir as mybir
import concourse.tile as tile
from concourse import bacc
from concourse.bass_utils import run_bass_kernel_spmd
from concourse.masks import make_identity

NCORES = 8
C, H, D, HID = 512, 8, 64, 1024
EPS = 1e-5
CHUNK_T = 16  # edge tiles (of 128 edges) per dma_gather chunk
F32 = mybir.dt.float32
BF16 = mybir.dt.bfloat16
I16 = mybir.dt.int16
AF = mybir.ActivationFunctionType
ALU = mybir.AluOpType
AX = mybir.AxisListType

_prog_cache = {}
TRACE = False          # set True (with the ntff hook registered) to profile
LAST_EXEC_NS = None    # exec time of the last run when TRACE was on
LAST_RESULTS = None    # full BassKernelResults of the last run


# --------------------------------------------------------------------------
# host-side preprocessing
# --------------------------------------------------------------------------

def _wrap_idx(idx):
    """[n] int -> [128, n//16] int16, wrapped (idx i at partition i%16,
    column i//16) and replicated across the 8 Q7 cores."""
    n = idx.shape[0]
    w = np.ascontiguousarray(idx.reshape(n // 16, 16).T).astype(np.int16)
    return np.tile(w, (8, 1))


def _preprocess_edges(L, row, col, att_bias):
    LSH = L // NCORES
    NBLK = LSH // 128
    bounds = np.searchsorted(row, np.arange(NCORES + 1) * LSH)

    per_core = []
    t_blk = 1
    for c in range(NCORES):
        e0, e1 = int(bounds[c]), int(bounds[c + 1])
        r = row[e0:e1] - c * LSH
        blk = r >> 7
        cnt = np.bincount(blk, minlength=NBLK)
        t_blk = max(t_blk, int(np.max((cnt + 127) // 128)) if len(cnt) else 1)
        per_core.append((e0, e1, r, blk, cnt))

    T_BLK = t_blk
    NT = NBLK * T_BLK
    NCH = (NT + CHUNK_T - 1) // CHUNK_T
    NTP = NCH * CHUNK_T  # tiles padded to whole chunks (extra tiles unused)

    cores = []
    for c in range(NCORES):
        e0, e1, r, blk, cnt = per_core[c]
        ne = e1 - e0
        starts = np.zeros(NBLK, dtype=np.int64)
        np.cumsum(cnt[:-1], out=starts[1:])
        idx_in_blk = np.arange(ne, dtype=np.int64) - starts[blk]
        dst = blk * (T_BLK * 128) + idx_in_blk

        npad = NTP * 128
        colP = np.zeros(npad, dtype=np.int64)
        qlocP = np.zeros(npad, dtype=np.int64)
        rlocP = np.zeros(npad, dtype=np.float32)
        biasP = np.full((npad, H), -30000.0, dtype=np.float32)
        colP[dst] = col[e0:e1]
        qlocP[dst] = r
        rlocP[dst] = (r & 127).astype(np.float32)
        biasP[dst] = att_bias[e0:e1]

        colw = _wrap_idx(colP).reshape(128, NCH, CHUNK_T * 8).transpose(1, 0, 2)
        qlocw = _wrap_idx(qlocP).reshape(128, NCH, CHUNK_T * 8).transpose(1, 0, 2)
        colw = colw.reshape(NCH * 128, CHUNK_T * 8)
        qlocw = qlocw.reshape(NCH * 128, CHUNK_T * 8)
        # [NT, 128, H] / [NT, 128] partition-major per tile
        biasT = biasP.reshape(NTP, 128, H)[:NT]
        rlocT = rlocP.reshape(NTP, 128)[:NT]
        cores.append(dict(
            colw=np.ascontiguousarray(colw),
            qlocw=np.ascontiguousarray(qlocw),
            biasP=np.ascontiguousarray(biasT),
            rlocP=np.ascontiguousarray(rlocT),
        ))
    return T_BLK, NT, NCH, cores


def _prep_weights(inp):
    scale = 1.0 / math.sqrt(D)

    def mat(w, kchunks):
        w = np.asarray(w, np.float32)
        k, n = w.shape
        assert k == kchunks * 128
        return np.ascontiguousarray(
            w.reshape(kchunks, 128, n).transpose(1, 0, 2)).astype(bfloat16)

    def rowv(b):
        return np.asarray(b, np.float32)[None, :].astype(bfloat16)

    return dict(
        wq=mat(np.asarray(inp["Wq"], np.float32) * scale, 4),
        wk=mat(inp["Wk"], 4),
        wv=mat(inp["Wv"], 4),
        wo=mat(inp["Wo"], 4),
        w1=mat(inp["W1"], 4),
        w2=mat(inp["W2"], 8),
        bq=rowv(np.asarray(inp["bq"], np.float32) * scale),
        bk=rowv(inp["bk"]), bv=rowv(inp["bv"]), bo=rowv(inp["bo"]),
        b1=rowv(inp["b1"]), b2=rowv(inp["b2"]),
        ln1g=np.asarray(inp["ln1_g"], np.float32),
        ln1b=np.asarray(inp["ln1_b"], np.float32),
        ln2g=np.asarray(inp["ln2_g"], np.float32),
        ln2b=np.asarray(inp["ln2_b"], np.float32),
    )


# --------------------------------------------------------------------------
# walrus workaround: this walrus build rejects Drain instructions carrying
# more than one sem wait ("Too many sync wait commands") -- split the extra
# waits onto NOPs inserted just before, on the same engine.
# --------------------------------------------------------------------------

def _split_multi_waits(nc):
    nid = [0]
    for fn in nc.m.functions:
        for blk in fn.blocks:
            insts = blk.instructions
            i = 0
            while i < len(insts):
                inst = insts[i]
                si = inst.sync_info
                if (isinstance(inst, mybir.InstDrain)
                        and si is not None and si.on_wait and len(si.on_wait) > 1):
                    waits = list(si.on_wait)
                    nops = []
                    for w in waits[:-1]:
                        nid[0] += 1
                        nops.append(mybir.InstNoOp(
                            name=f"I-waitfix-{nid[0]}",
                            engine=inst.engine, ins=[], outs=[],
                            sync_info=mybir.SyncInfo(on_wait=[w], on_update=[]),
                        ))
                    inst.sync_info = mybir.SyncInfo(
                        on_wait=[waits[-1]], on_update=list(si.on_update))
                    insts[i:i] = nops
                    i += len(nops)
                i += 1


# --------------------------------------------------------------------------
# device program
# --------------------------------------------------------------------------

def _bc(ap, n):
    """append a broadcast (step-0) innermost dim of size n to an AP"""
    return bass.AP(tensor=ap.tensor, offset=ap.offset, ap=[*ap.ap, [0, n]])


def _phd(ap):
    return ap.rearrange("p (h d) -> p h d", h=H)


def _build_program(L, T_BLK, NT, NCH):
    LSH = L // NCORES
    NBLK = LSH // 128
    nc = bacc.Bacc(num_devices=NCORES)

    x_c = nc.declare_dram_parameter("x_c", [LSH, C], F32, isOutput=False)
    wq = nc.declare_dram_parameter("wq", [128, 4, C], BF16, isOutput=False)
    wk = nc.declare_dram_parameter("wk", [128, 4, C], BF16, isOutput=False)
    wv = nc.declare_dram_parameter("wv", [128, 4, C], BF16, isOutput=False)
    wo = nc.declare_dram_parameter("wo", [128, 4, C], BF16, isOutput=False)
    w1 = nc.declare_dram_parameter("w1", [128, 4, HID], BF16, isOutput=False)
    w2 = nc.declare_dram_parameter("w2", [128, 8, C], BF16, isOutput=False)
    bqp = nc.declare_dram_parameter("bq", [1, C], BF16, isOutput=False)
    bkp = nc.declare_dram_parameter("bk", [1, C], BF16, isOutput=False)
    bvp = nc.declare_dram_parameter("bv", [1, C], BF16, isOutput=False)
    bop = nc.declare_dram_parameter("bo", [1, C], BF16, isOutput=False)
    b1p = nc.declare_dram_parameter("b1", [1, HID], BF16, isOutput=False)
    b2p = nc.declare_dram_parameter("b2", [1, C], BF16, isOutput=False)
    ln1g = nc.declare_dram_parameter("ln1g", [C], F32, isOutput=False)
    ln1b = nc.declare_dram_parameter("ln1b", [C], F32, isOutput=False)
    ln2g = nc.declare_dram_parameter("ln2g", [C], F32, isOutput=False)
    ln2b = nc.declare_dram_parameter("ln2b", [C], F32, isOutput=False)
    colw = nc.declare_dram_parameter("colw", [NCH * 128, CHUNK_T * 8], I16, isOutput=False)
    qlocw = nc.declare_dram_parameter("qlocw", [NCH * 128, CHUNK_T * 8], I16, isOutput=False)
    biasP = nc.declare_dram_parameter("biasP", [NT, 128, H], F32, isOutput=False)
    rlocP = nc.declare_dram_parameter("rlocP", [NT, 128], F32, isOutput=False)
    y_out = nc.declare_dram_parameter("y", [LSH, C], F32, isOutput=True)

    with ExitStack() as ctx:
        tc = ctx.enter_context(tile.TileContext(nc))

        dram = ctx.enter_context(tc.tile_pool(name="dram", bufs=1, space="DRAM"))
        q_tab = dram.tile([LSH, C], BF16)
        kv_sh = dram.tile([LSH, 2 * C], BF16)
        kv_full = dram.tile([NCORES * LSH, 2 * C], BF16)
        x1_d = dram.tile([LSH, C], F32)

        # ---------------- constants + weights ----------------
        consts = ctx.enter_context(tc.tile_pool(name="consts", bufs=1))
        ident = consts.tile([128, 128], BF16, tag="ident")
        make_identity(nc, ident[:])
        iota_row = consts.tile([128, 128], BF16, tag="iota")
        nc.gpsimd.iota(iota_row[:], pattern=[[1, 128]], base=0,
                       channel_multiplier=0,
                       allow_small_or_imprecise_dtypes=True)
        ones_k1 = consts.tile([1, 128], BF16, tag="ones")
        nc.vector.memset(ones_k1[:], 1.0)
        eps_t = consts.tile([128, 1], F32, tag="eps")
        nc.vector.memset(eps_t[:], EPS)

        def bcast_load(param, tag):
            t = consts.tile([128, C], F32, tag=tag)
            ap = param[:]
            src = bass.AP(tensor=ap.tensor, offset=ap.offset,
                          ap=[[0, 128], [1, C]])
            nc.sync.dma_start(out=t[:], in_=src)
            return t

        g1_bc, b1_bc = bcast_load(ln1g, "g1"), bcast_load(ln1b, "b1")
        g2_bc, b2_bc = bcast_load(ln2g, "g2"), bcast_load(ln2b, "b2")

        wts = ctx.enter_context(tc.tile_pool(name="wts", bufs=1))

        def wload(p, shape, tag):
            t = wts.tile(shape, BF16, tag=tag)
            nc.sync.dma_start(out=t[:], in_=p[:])
            return t

        wq_sb = wload(wq, [128, 4, C], "wq"); wk_sb = wload(wk, [128, 4, C], "wk")
        wv_sb = wload(wv, [128, 4, C], "wv"); wo_sb = wload(wo, [128, 4, C], "wo")
        w1_sb = wload(w1, [128, 4, HID], "w1"); w2_sb = wload(w2, [128, 8, C], "w2")
        bq_sb = wload(bqp, [1, C], "bq"); bk_sb = wload(bkp, [1, C], "bk")
        bv_sb = wload(bvp, [1, C], "bv"); bo_sb = wload(bop, [1, C], "bo")
        b1_sb = wload(b1p, [1, HID], "bb1"); b2_sb = wload(b2p, [1, C], "bb2")

        # ---------------- LN helper ----------------
        def layernorm(pool, lnpool, xb, g_bc, bb_bc):
            """returns bf16 [128, C] normalized tile"""
            stats = lnpool.tile([128, 6], F32, tag="stats")
            nc.vector.bn_stats(stats[:], xb[:])
            mv = lnpool.tile([128, 2], F32, tag="mv")
            nc.vector.bn_aggr(mv[:], stats[:])
            xc = pool.tile([128, C], F32, tag="ln_xc")
            nc.vector.tensor_scalar(xc[:], xb[:], mv[:, 0:1], None, op0=ALU.subtract)
            sd = lnpool.tile([128, 1], F32, tag="sd")
            nc.scalar.activation(sd[:], mv[:, 1:2], AF.Sqrt, bias=eps_t[:])
            rstd = lnpool.tile([128, 1], F32, tag="rstd")
            nc.vector.reciprocal(rstd[:], sd[:])
            z0 = pool.tile([128, C], F32, tag="ln_z0")
            nc.vector.tensor_scalar(z0[:], xc[:], rstd[:], None, op0=ALU.mult)
            z1 = pool.tile([128, C], F32, tag="ln_z1")
            nc.vector.tensor_mul(z1[:], z0[:], g_bc[:])
            zb = pool.tile([128, C], BF16, tag="ln_out")
            nc.vector.tensor_add(zb[:], z1[:], bb_bc[:])
            return zb

        # ---------------- phase B+C: LN1, zT, QKV ----------------
        with ExitStack() as pctx:
            zT_pool = pctx.enter_context(tc.tile_pool(name="zT", bufs=1))
            zT = zT_pool.tile([128, 4, LSH], BF16)
            xp = pctx.enter_context(tc.tile_pool(name="xp", bufs=3))
            lnp = pctx.enter_context(tc.tile_pool(name="lnp", bufs=4))
            trp = pctx.enter_context(tc.tile_pool(name="trp", bufs=2, space="PSUM"))
            qkvp = pctx.enter_context(tc.tile_pool(name="qkvp", bufs=2, space="PSUM"))
            obp = pctx.enter_context(tc.tile_pool(name="obp", bufs=3))

            for ib in range(NBLK):
                sl = slice(ib * 128, (ib + 1) * 128)
                xb = xp.tile([128, C], F32, tag="xin")
                nc.sync.dma_start(out=xb[:], in_=x_c[sl, :])
                zb = layernorm(xp, lnp, xb, g1_bc, b1_bc)
                for g in range(4):
                    pt = trp.tile([128, 128], BF16)
                    nc.tensor.transpose(pt[:], zb[:, g * 128:(g + 1) * 128], ident[:])
                    nc.scalar.copy(zT[:, g, sl], pt[:])
                for w_sb, bias_sb, dst in (
                    (wq_sb, bq_sb, None),
                    (wk_sb, bk_sb, 0),
                    (wv_sb, bv_sb, 1),
                ):
                    ps = qkvp.tile([128, C], F32)
                    for g in range(4):
                        nc.tensor.matmul(ps[:], lhsT=zT[:, g, sl], rhs=w_sb[:, g, :],
                                         start=(g == 0), stop=False)
                    nc.tensor.matmul(ps[:], lhsT=ones_k1[:], rhs=bias_sb[:],
                                     start=False, stop=True)
                    ob = obp.tile([128, C], BF16)
                    nc.scalar.copy(ob[:], ps[:])
                    if dst is None:
                        nc.sync.dma_start(out=q_tab[sl, :], in_=ob[:])
                    else:
                        nc.sync.dma_start(out=kv_sh[sl, dst * C:(dst + 1) * C], in_=ob[:])

        # ---------------- phase D: allgather K/V ----------------
        nc.gpsimd.collective_compute(
            "AllGather", ALU.bypass,
            replica_groups=[list(range(NCORES))],
            ins=[kv_sh[:]], outs=[kv_full[:]],
        )

        # ---------------- phase E: edges ----------------
        with ExitStack() as pctx:
            kvp = pctx.enter_context(tc.tile_pool(name="kvp", bufs=2))
            qgp = pctx.enter_context(tc.tile_pool(name="qgp", bufs=2))
            idxp = pctx.enter_context(tc.tile_pool(name="idxp", bufs=3))
            bp = pctx.enter_context(tc.tile_pool(name="bp", bufs=2))
            rlp = pctx.enter_context(tc.tile_pool(name="rlp", bufs=2))
            work = pctx.enter_context(tc.tile_pool(name="work", bufs=4))
            pop_ = pctx.enter_context(tc.tile_pool(name="pout", bufs=2, space="PSUM"))
            psp = pctx.enter_context(tc.tile_pool(name="pssum", bufs=1, space="PSUM"))
            trp2 = pctx.enter_context(tc.tile_pool(name="trp2", bufs=2, space="PSUM"))
            opp = pctx.enter_context(tc.tile_pool(name="opsum", bufs=1, space="PSUM"))
            finp = pctx.enter_context(tc.tile_pool(name="finp", bufs=2))

            kvb = qgb = bia = rlc = None
            pout = pssum = None
            for t in range(NT):
                ch, slot = divmod(t, CHUNK_T)
                if slot == 0:
                    tiles_c = min(CHUNK_T, NT - ch * CHUNK_T)
                    n_idx = tiles_c * 128
                    cidx = idxp.tile([128, CHUNK_T * 8], I16, tag="cidx")
                    nc.sync.dma_start(out=cidx[:], in_=colw[ch * 128:(ch + 1) * 128, :])
                    qidx = idxp.tile([128, CHUNK_T * 8], I16, tag="qidx")
                    nc.sync.dma_start(out=qidx[:], in_=qlocw[ch * 128:(ch + 1) * 128, :])
                    kvb = kvp.tile([128, CHUNK_T, 2 * C], BF16)
                    nc.gpsimd.dma_gather(
                        out_ap=kvb[:, :tiles_c, :], in_ap=kv_full[:],
                        idxs_ap=cidx[:, :n_idx // 16],
                        num_idxs=n_idx, num_idxs_reg=n_idx, elem_size=2 * C,
                        single_packet=False)
                    qgb = qgp.tile([128, CHUNK_T, C], BF16)
                    nc.gpsimd.dma_gather(
                        out_ap=qgb[:, :tiles_c, :], in_ap=q_tab[:],
                        idxs_ap=qidx[:, :n_idx // 16],
                        num_idxs=n_idx, num_idxs_reg=n_idx, elem_size=C,
                        single_packet=False)
                    bia = bp.tile([128, CHUNK_T, H], F32)
                    nc.sync.dma_start(
                        out=bia[:, :tiles_c, :],
                        in_=biasP[ch * CHUNK_T:ch * CHUNK_T + tiles_c, :, :]
                        .rearrange("t p h -> p t h"))
                    rlc = rlp.tile([128, CHUNK_T], F32)
                    nc.sync.dma_start(
                        out=rlc[:, :tiles_c],
                        in_=rlocP[ch * CHUNK_T:ch * CHUNK_T + tiles_c, :]
                        .rearrange("t p -> p t"))

                rb, tb = divmod(t, T_BLK)
                if tb == 0:
                    pout = pop_.tile([128, C], F32)
                    pssum = psp.tile([128, H], F32)

                kg = kvb[:, slot, 0:C]
                vg = kvb[:, slot, C:2 * C]
                qg = qgb[:, slot, :]
                prod = work.tile([128, C], BF16, tag="prod")
                nc.vector.tensor_mul(prod[:], kg, qg)
                sc = work.tile([128, H], F32, tag="sc")
                nc.vector.tensor_reduce(sc[:], _phd(prod[:]), axis=AX.X, op=ALU.add)
                sc2 = work.tile([128, H], F32, tag="sc2")
                nc.vector.tensor_add(sc2[:], sc[:], bia[:, slot, :])
                p_t = work.tile([128, H], BF16, tag="p")
                nc.scalar.activation(p_t[:], sc2[:], AF.Exp)
                oh = work.tile([128, 128], BF16, tag="oh")
                nc.vector.tensor_scalar(oh[:], iota_row[:], rlc[:, slot:slot + 1],
                                        None, op0=ALU.is_equal)
                wt = work.tile([128, C], BF16, tag="wt")
                nc.vector.tensor_tensor(_phd(wt[:]), _phd(vg), _bc(p_t[:], D),
                                        op=ALU.mult)
                nc.tensor.matmul(pout[:], lhsT=oh[:], rhs=wt[:],
                                 start=(tb == 0), stop=(tb == T_BLK - 1))
                nc.tensor.matmul(pssum[:], lhsT=oh[:], rhs=p_t[:],
                                 start=(tb == 0), stop=(tb == T_BLK - 1))

                if tb == T_BLK - 1:
                    sl = slice(rb * 128, (rb + 1) * 128)
                    sm = finp.tile([128, H], F32, tag="sm")
                    nc.vector.tensor_scalar(sm[:], pssum[:], 1e-30, None, op0=ALU.max)
                    rec = finp.tile([128, H], F32, tag="rec")
                    nc.vector.reciprocal(rec[:], sm[:])
                    att = finp.tile([128, C], BF16, tag="att")
                    nc.vector.tensor_tensor(_phd(att[:]), _phd(pout[:]),
                                            _bc(rec[:], D), op=ALU.mult)
                    attT = finp.tile([128, 4, 128], BF16, tag="attT")
                    for g in range(4):
                        pt = trp2.tile([128, 128], BF16)
                        nc.tensor.transpose(pt[:], att[:, g * 128:(g + 1) * 128], ident[:])
                        nc.scalar.copy(attT[:, g, :], pt[:])
                    po = opp.tile([128, C], F32)
                    for g in range(4):
                        nc.tensor.matmul(po[:], lhsT=attT[:, g, :], rhs=wo_sb[:, g, :],
                                         start=(g == 0), stop=False)
                    nc.tensor.matmul(po[:], lhsT=ones_k1[:], rhs=bo_sb[:],
                                     start=False, stop=True)
                    xb2 = finp.tile([128, C], F32, tag="xb2")
                    nc.sync.dma_start(out=xb2[:], in_=x_c[sl, :])
                    x1t = finp.tile([128, C], F32, tag="x1t")
                    nc.vector.tensor_add(x1t[:], po[:], xb2[:])
                    nc.sync.dma_start(out=x1_d[sl, :], in_=x1t[:])

        # ---------------- phase F: LN2 + MLP ----------------
        with ExitStack() as pctx:
            xp = pctx.enter_context(tc.tile_pool(name="xp2", bufs=3))
            lnp = pctx.enter_context(tc.tile_pool(name="lnp2", bufs=4))
            trp3 = pctx.enter_context(tc.tile_pool(name="trp3", bufs=2, space="PSUM"))
            hp = pctx.enter_context(tc.tile_pool(name="hpsum", bufs=1, space="PSUM"))
            yp = pctx.enter_context(tc.tile_pool(name="ypsum", bufs=1, space="PSUM"))
            sbp = pctx.enter_context(tc.tile_pool(name="sbp", bufs=3))

            for ib in range(NBLK):
                sl = slice(ib * 128, (ib + 1) * 128)
                x1t = xp.tile([128, C], F32, tag="x1in")
                nc.sync.dma_start(out=x1t[:], in_=x1_d[sl, :])
                z2 = layernorm(xp, lnp, x1t, g2_bc, b2_bc)
                z2T = sbp.tile([128, 4, 128], BF16, tag="z2T")
                for g in range(4):
                    pt = trp3.tile([128, 128], BF16)
                    nc.tensor.transpose(pt[:], z2[:, g * 128:(g + 1) * 128], ident[:])
                    nc.scalar.copy(z2T[:, g, :], pt[:])
                ph = hp.tile([128, 8, 128], F32)
                for chc in range(8):
                    csl = slice(chc * 128, (chc + 1) * 128)
                    for g in range(4):
                        nc.tensor.matmul(ph[:, chc, :], lhsT=w1_sb[:, g, csl],
                                         rhs=z2T[:, g, :], start=(g == 0), stop=False)
                    nc.tensor.matmul(ph[:, chc, :], lhsT=b1_sb[:, csl],
                                     rhs=ones_k1[:], start=False, stop=True)
                hs = sbp.tile([128, 8, 128], BF16, tag="hs")
                nc.scalar.activation(hs[:], ph[:], AF.Silu)
                py = yp.tile([128, C], F32)
                for chc in range(8):
                    nc.tensor.matmul(py[:], lhsT=hs[:, chc, :], rhs=w2_sb[:, chc, :],
                                     start=(chc == 0), stop=False)
                nc.tensor.matmul(py[:], lhsT=ones_k1[:], rhs=b2_sb[:],
                                 start=False, stop=True)
                yt = sbp.tile([128, C], F32, tag="yt")
                nc.vector.tensor_add(yt[:], py[:], x1t[:])
                nc.sync.dma_start(out=y_out[sl, :], in_=yt[:])

    nc.finalize()
    _split_multi_waits(nc)
    return nc


# --------------------------------------------------------------------------
# entry point
# --------------------------------------------------------------------------

def kernel(**inputs) -> np.ndarray:
    x = np.asarray(inputs["x"], np.float32)
    row = np.asarray(inputs["row_index"]).astype(np.int64)
    col = np.asarray(inputs["col_index"]).astype(np.int64)
    att_bias = np.asarray(inputs["att_bias"], np.float32)
    L = x.shape[0]
    LSH = L // NCORES

    T_BLK, NT, NCH, cores = _preprocess_edges(L, row, col, att_bias)
    w = _prep_weights(inputs)

    key = (L, T_BLK, NT, NCH)
    if key not in _prog_cache:
        _prog_cache[key] = _build_program(L, T_BLK, NT, NCH)
    nc = _prog_cache[key]

    in_maps = []
    for c in range(NCORES):
        m = dict(w)
        m["x_c"] = np.ascontiguousarray(x[c * LSH:(c + 1) * LSH])
        m.update(cores[c])
        in_maps.append(m)

    global LAST_EXEC_NS, LAST_RESULTS
    res = run_bass_kernel_spmd(nc, in_maps, list(range(NCORES)), trace=TRACE)
    LAST_RESULTS = res
    LAST_EXEC_NS = res.exec_time_ns
    return np.concatenate([res.results[c]["y"] for c in range(NCORES)], axis=0)



# revision 5
# speedup vs baseline: 1.0913x; 1.0913x over previous
"""Trainium2 Bass kernel v3 for the sparse-attention EncoderLayer.

Design (per core, rows sharded 8 ways, edges owned by dest-row core):
- KV table rows are [K fp8 512B | V fp8 512B] = 1024 B; one dma_gather per
  edge fetches both.  AllGather of the table is chunked into 4 row-quarters
  kicked as soon as each quarter's K/V is projected (overlaps the QKV phase).
- Q never leaves SBUF; per-edge Q rows come from a PE matmul
  Qe = ohT @ Qblk with host-shipped fp8 one-hot matrices.
- Scores: custom DVE scan op (fused multiply + segmented prefix sum over the
  8x64 head groups) whose output AP has a step-0 inner dim, so only the
  per-head totals are materialized ([128, 8] per tile).  Bias add + exp are
  batched per 16-tile chunk (exp on the Scalar engine, one op per chunk).
- wt = v * p: DVE 3D-broadcast multiply, optionally alternated onto GPSIMD
  (WT_SPLIT env var) to balance engines.
- Scatter per 128-row block via one-hot PE matmuls accumulating in PSUM.
- LayerNorm gamma/beta folded into the adjacent projection weights on the
  host; device LN is bn_stats/bn_aggr + a single tensor_scalar.
- LN2+MLP fused into the edge phase per row-block.
"""

import math
import os
import numpy as np
from contextlib import ExitStack

from ml_dtypes import bfloat16, float8_e4m3

import concourse.bass as bass
import concourse.mybir as mybir
import concourse.tile as tile
from concourse import bacc
from concourse.bass_utils import run_bass_kernel_spmd
from concourse.masks import make_identity

NCORES = 8
C, H, D, HID = 512, 8, 64, 1024
EPS = 1e-5
CHUNK_T = 16
KB = 512           # K bytes per row (fp8)
EB = 1024          # fused row bytes (K fp8 + V fp8)
F32 = mybir.dt.float32
BF16 = mybir.dt.bfloat16
FP8 = mybir.dt.float8e4
I16 = mybir.dt.int16
AF = mybir.ActivationFunctionType
ALU = mybir.AluOpType
AX = mybir.AxisListType

WT_SPLIT = float(os.environ.get("WT_SPLIT", "0.0"))  # fraction of wt on gpsimd

_prog_cache = {}
TRACE = False
LAST_EXEC_NS = None
LAST_RESULTS = None


# --------------------------------------------------------------------------
# custom DVE op: cumsum(in0 * in1) along free dim (f32 state); with
# subdim=True the [P, S, N] shape resets the sum at each S boundary.
# --------------------------------------------------------------------------

def _register_scan(subdim):
    import concourse.dve_ops as dve_ops
    from concourse.dve_spec import Spec, Src0, Src1, scan, lower, AluOp
    from concourse.dve_uop import DveOpSpec

    name = "KQ_SCAN_MUL_SD" if subdim else "KQ_SCAN_MUL"
    for o in dve_ops.OPS:
        if o.name == name:
            return o

    def ref(in0, in1, s0, s1, imm2):
        p0 = in0.astype(np.float32)
        p1 = in1.astype(np.float32)
        prod = (p0 * p1).reshape(p0.shape[0], -1)
        return np.cumsum(prod, axis=-1).reshape(p0.shape).astype(np.float32)

    spec = Spec(body=scan(AluOp.ADD, Src0 * Src1), reference=ref)
    row = dve_ops._CUSTOM_DVE_ROW_BASE + len(dve_ops.OPS)
    assert row < 0x20
    dve_ops._SUB_OPCODE_FOR_NAME[name] = row
    shas = {}
    for ver in ("v3", "v4"):
        uops = lower(spec, ver=ver)
        shas[ver] = DveOpSpec(name=name, opcode=row, uops=uops, rd1_en=True).sha(ver)
    op = dve_ops.DveOp(name, spec, subdim=subdim, uops_sha=shas)
    dve_ops.OPS.append(op)
    dve_ops.CUSTOM_DVE_SPECS[name] = spec
    return op


# --------------------------------------------------------------------------
# host-side preprocessing
# --------------------------------------------------------------------------

def _wrap_idx(idx):
    n = idx.shape[0]
    w = np.ascontiguousarray(idx.reshape(n // 16, 16).T).astype(np.int16)
    return np.tile(w, (8, 1))


def _remap_col(col, L):
    """kv_full row order is (quarter, core, local_within_quarter)."""
    LSH = L // NCORES
    NBLK = LSH // 128
    nq = 4 if NBLK % 4 == 0 else 1
    QS = LSH // nq
    core = col // LSH
    rem = col - core * LSH
    quarter = rem // QS
    local = rem - quarter * QS
    return quarter * (NCORES * QS) + core * QS + local


def _preprocess_edges(L, row, col, att_bias):
    LSH = L // NCORES
    NBLK = LSH // 128
    bounds = np.searchsorted(row, np.arange(NCORES + 1) * LSH)
    colm = _remap_col(col, L)

    per_core = []
    t_blk = 1
    for c in range(NCORES):
        e0, e1 = int(bounds[c]), int(bounds[c + 1])
        r = row[e0:e1] - c * LSH
        blk = r >> 7
        cnt = np.bincount(blk, minlength=NBLK)
        t_blk = max(t_blk, int(np.max((cnt + 127) // 128)) if len(cnt) else 1)
        per_core.append((e0, e1, r, blk, cnt))

    T_BLK = t_blk
    NT = NBLK * T_BLK
    NCH = (NT + CHUNK_T - 1) // CHUNK_T
    NTP = NCH * CHUNK_T

    ar = np.arange(128, dtype=np.int64)
    cores = []
    for c in range(NCORES):
        e0, e1, r, blk, cnt = per_core[c]
        ne = e1 - e0
        starts = np.zeros(NBLK, dtype=np.int64)
        np.cumsum(cnt[:-1], out=starts[1:])
        idx_in_blk = np.arange(ne, dtype=np.int64) - starts[blk]
        dst = blk * (T_BLK * 128) + idx_in_blk

        npad = NTP * 128
        colP = np.zeros(npad, dtype=np.int64)
        rlocP = np.zeros(npad, dtype=np.int64)
        biasP = np.full((npad, H), -30000.0, dtype=np.float32)
        colP[dst] = colm[e0:e1]
        rlocP[dst] = r & 127
        biasP[dst] = att_bias[e0:e1]

        colw = _wrap_idx(colP).reshape(128, NCH, CHUNK_T * 8).transpose(1, 0, 2)
        colw = np.ascontiguousarray(colw.reshape(NCH * 128, CHUNK_T * 8))

        rT = rlocP.reshape(NTP, 128)                  # [t, e]
        oh = (rT[:, :, None] == ar[None, None, :])    # [t, e, r]
        ohP = np.ascontiguousarray(
            oh.transpose(1, 0, 2).reshape(128, NTP * 128)).astype(float8_e4m3)
        ohT = np.ascontiguousarray(
            oh.transpose(2, 0, 1).reshape(128, NTP * 128)).astype(float8_e4m3)
        biasT = np.ascontiguousarray(
            biasP.reshape(NTP, 128, H).transpose(1, 0, 2)
            .reshape(128, NTP * H)).astype(np.float32)
        cores.append(dict(colw=colw, ohP=ohP, ohT=ohT, biasT=biasT))
    return T_BLK, NT, NCH, cores


def _prep_weights(inp):
    scale = 1.0 / math.sqrt(D)
    g1 = np.asarray(inp["ln1_g"], np.float32)
    b1n = np.asarray(inp["ln1_b"], np.float32)
    g2 = np.asarray(inp["ln2_g"], np.float32)
    b2n = np.asarray(inp["ln2_b"], np.float32)

    def mat(w, kchunks):
        w = np.asarray(w, np.float32)
        k, n = w.shape
        assert k == kchunks * 128
        return np.ascontiguousarray(
            w.reshape(kchunks, 128, n).transpose(1, 0, 2)).astype(bfloat16)

    def rowv(b):
        return np.asarray(b, np.float32)[None, :].astype(bfloat16)

    Wq = np.asarray(inp["Wq"], np.float32)
    Wk = np.asarray(inp["Wk"], np.float32)
    Wv = np.asarray(inp["Wv"], np.float32)
    W1 = np.asarray(inp["W1"], np.float32)
    # fold LN gains/biases into the projections
    Wq_e = (g1[:, None] * Wq) * scale
    bq_e = (np.asarray(inp["bq"], np.float32) + b1n @ Wq) * scale
    Wk_e = g1[:, None] * Wk
    bk_e = np.asarray(inp["bk"], np.float32) + b1n @ Wk
    Wv_e = g1[:, None] * Wv
    bv_e = np.asarray(inp["bv"], np.float32) + b1n @ Wv
    W1_e = g2[:, None] * W1
    b1_e = np.asarray(inp["b1"], np.float32) + b2n @ W1

    return dict(
        wq=mat(Wq_e, 4), wk=mat(Wk_e, 4), wv=mat(Wv_e, 4),
        wo=mat(inp["Wo"], 4), w1=mat(W1_e, 4), w2=mat(inp["W2"], 8),
        bq=rowv(bq_e), bk=rowv(bk_e), bv=rowv(bv_e),
        bo=rowv(inp["bo"]), b1=rowv(b1_e), b2=rowv(inp["b2"]),
    )


# --------------------------------------------------------------------------
# walrus workaround (drains with >1 sem wait)
# --------------------------------------------------------------------------

def _split_multi_waits(nc):
    nid = [0]
    for fn in nc.m.functions:
        for blk in fn.blocks:
            insts = blk.instructions
            i = 0
            while i < len(insts):
                inst = insts[i]
                si = inst.sync_info
                if (isinstance(inst, mybir.InstDrain)
                        and si is not None and si.on_wait and len(si.on_wait) > 1):
                    waits = list(si.on_wait)
                    nops = []
                    for w in waits[:-1]:
                        nid[0] += 1
                        nops.append(mybir.InstNoOp(
                            name=f"I-waitfix-{nid[0]}",
                            engine=inst.engine, ins=[], outs=[],
                            sync_info=mybir.SyncInfo(on_wait=[w], on_update=[]),
                        ))
                    inst.sync_info = mybir.SyncInfo(
                        on_wait=[waits[-1]], on_update=list(si.on_update))
                    insts[i:i] = nops
                    i += len(nops)
                i += 1


# --------------------------------------------------------------------------
# device program
# --------------------------------------------------------------------------

def _bc(ap, n):
    return bass.AP(tensor=ap.tensor, offset=ap.offset, ap=[*ap.ap, [0, n]])


def _phd(ap):
    return ap.rearrange("p (h d) -> p h d", h=H)


def _build_program(L, T_BLK, NT, NCH):
    scan_op = _register_scan(False)
    LSH = L // NCORES
    NBLK = LSH // 128
    QS = LSH // 4
    nc = bacc.Bacc(num_devices=NCORES)

    x_c = nc.declare_dram_parameter("x_c", [LSH, C], F32, isOutput=False)
    wq = nc.declare_dram_parameter("wq", [128, 4, C], BF16, isOutput=False)
    wk = nc.declare_dram_parameter("wk", [128, 4, C], BF16, isOutput=False)
    wv = nc.declare_dram_parameter("wv", [128, 4, C], BF16, isOutput=False)
    wo = nc.declare_dram_parameter("wo", [128, 4, C], BF16, isOutput=False)
    w1 = nc.declare_dram_parameter("w1", [128, 4, HID], BF16, isOutput=False)
    w2 = nc.declare_dram_parameter("w2", [128, 8, C], BF16, isOutput=False)
    bqp = nc.declare_dram_parameter("bq", [1, C], BF16, isOutput=False)
    bkp = nc.declare_dram_parameter("bk", [1, C], BF16, isOutput=False)
    bvp = nc.declare_dram_parameter("bv", [1, C], BF16, isOutput=False)
    bop = nc.declare_dram_parameter("bo", [1, C], BF16, isOutput=False)
    b1p = nc.declare_dram_parameter("b1", [1, HID], BF16, isOutput=False)
    b2p = nc.declare_dram_parameter("b2", [1, C], BF16, isOutput=False)
    colw = nc.declare_dram_parameter("colw", [NCH * 128, CHUNK_T * 8], I16, isOutput=False)
    ohP_d = nc.declare_dram_parameter("ohP", [128, NCH * CHUNK_T * 128], FP8, isOutput=False)
    ohT_d = nc.declare_dram_parameter("ohT", [128, NCH * CHUNK_T * 128], FP8, isOutput=False)
    biasT_d = nc.declare_dram_parameter("biasT", [128, NCH * CHUNK_T * H], F32, isOutput=False)
    y_out = nc.declare_dram_parameter("y", [LSH, C], F32, isOutput=True)

    with ExitStack() as ctx:
        tc = ctx.enter_context(tile.TileContext(nc))

        dram = ctx.enter_context(tc.tile_pool(name="dram", bufs=1, space="DRAM"))
        kv_sh = dram.tile([LSH, EB], FP8)
        kv_full = dram.tile([NCORES * LSH, EB], FP8)

        # ---------------- constants + weights ----------------
        consts = ctx.enter_context(tc.tile_pool(name="consts", bufs=1))
        ident = consts.tile([128, 128], BF16, tag="ident")
        make_identity(nc, ident[:])
        ones_k1 = consts.tile([1, 128], BF16, tag="ones")
        nc.vector.memset(ones_k1[:], 1.0)
        eps_t = consts.tile([128, 1], F32, tag="eps")
        nc.vector.memset(eps_t[:], EPS)

        wts = ctx.enter_context(tc.tile_pool(name="wts", bufs=1))

        def wload(p, shape, tag):
            t = wts.tile(shape, BF16, tag=tag)
            nc.sync.dma_start(out=t[:], in_=p[:])
            return t

        wq_sb = wload(wq, [128, 4, C], "wq"); wk_sb = wload(wk, [128, 4, C], "wk")
        wv_sb = wload(wv, [128, 4, C], "wv"); wo_sb = wload(wo, [128, 4, C], "wo")
        w1_sb = wload(w1, [128, 4, HID], "w1"); w2_sb = wload(w2, [128, 8, C], "w2")
        bq_sb = wload(bqp, [1, C], "bq"); bk_sb = wload(bkp, [1, C], "bk")
        bv_sb = wload(bvp, [1, C], "bv"); bo_sb = wload(bop, [1, C], "bo")
        b1_sb = wload(b1p, [1, HID], "bb1"); b2_sb = wload(b2p, [1, C], "bb2")

        qtab = ctx.enter_context(tc.tile_pool(name="qtab", bufs=1))
        q_sb = qtab.tile([128, NBLK, C], BF16)

        # ---------------- LN helper (gamma/beta folded into weights) -----
        def layernorm(pool, lnpool, xb):
            """returns bf16 [128, C] (x - mean) * rstd"""
            stats = lnpool.tile([128, 6], F32, tag="stats")
            nc.vector.bn_stats(stats[:], xb[:])
            mv = lnpool.tile([128, 2], F32, tag="mv")
            nc.vector.bn_aggr(mv[:], stats[:])
            sd = lnpool.tile([128, 1], F32, tag="sd")
            nc.scalar.activation(sd[:], mv[:, 1:2], AF.Sqrt, bias=eps_t[:])
            rstd = lnpool.tile([128, 1], F32, tag="rstd")
            nc.vector.reciprocal(rstd[:], sd[:])
            xc = pool.tile([128, C], F32, tag="ln_xc")
            nc.vector.tensor_scalar(xc[:], xb[:], mv[:, 0:1], None, op0=ALU.subtract)
            zb = pool.tile([128, C], BF16, tag="ln_out")
            nc.vector.tensor_scalar(zb[:], xc[:], rstd[:], None, op0=ALU.mult)
            return zb

        # ---------------- phase B: LN1 + K/V (+chunked AG) + Q ----------
        with ExitStack() as pctx:
            zT_pool = pctx.enter_context(tc.tile_pool(name="zT", bufs=1))
            zT = zT_pool.tile([128, 4, LSH], BF16)
            xp = pctx.enter_context(tc.tile_pool(name="xp", bufs=3))
            lnp = pctx.enter_context(tc.tile_pool(name="lnp", bufs=4))
            trp = pctx.enter_context(tc.tile_pool(name="trp", bufs=2, space="PSUM"))
            kvp = pctx.enter_context(tc.tile_pool(name="kvpsum", bufs=2, space="PSUM"))
            obp = pctx.enter_context(tc.tile_pool(name="obp", bufs=3))

            with nc.named_scope("ln1_kv"):
                for ib in range(NBLK):
                    sl = slice(ib * 128, (ib + 1) * 128)
                    xb = xp.tile([128, C], F32, tag="xin")
                    nc.sync.dma_start(out=xb[:], in_=x_c[sl, :])
                    zb = layernorm(xp, lnp, xb)
                    for g in range(4):
                        pt = trp.tile([128, 128], BF16)
                        nc.tensor.transpose(pt[:], zb[:, g * 128:(g + 1) * 128], ident[:])
                        nc.scalar.copy(zT[:, g, sl], pt[:])
                    ps = kvp.tile([128, 2 * C], F32)
                    for g in range(4):
                        nc.tensor.matmul(ps[:, 0:C], lhsT=zT[:, g, sl],
                                         rhs=wk_sb[:, g, :], start=(g == 0), stop=False)
                    nc.tensor.matmul(ps[:, 0:C], lhsT=ones_k1[:], rhs=bk_sb[:],
                                     start=False, stop=True)
                    for g in range(4):
                        nc.tensor.matmul(ps[:, C:2 * C], lhsT=zT[:, g, sl],
                                         rhs=wv_sb[:, g, :], start=(g == 0), stop=False)
                    nc.tensor.matmul(ps[:, C:2 * C], lhsT=ones_k1[:], rhs=bv_sb[:],
                                     start=False, stop=True)
                    ob = obp.tile([128, 2 * C], FP8, tag="obkv")
                    nc.scalar.copy(ob[:], ps[:])
                    nc.sync.dma_start(out=kv_sh[sl, :], in_=ob[:])
                    nq = 4 if NBLK % 4 == 0 else 1
                    if (ib + 1) % (NBLK // nq) == 0:
                        qtr = (ib + 1) // (NBLK // nq) - 1
                        qs = LSH // nq
                        nc.gpsimd.collective_compute(
                            "AllGather", ALU.bypass,
                            replica_groups=[list(range(NCORES))],
                            ins=[kv_sh[qtr * qs:(qtr + 1) * qs, :]],
                            outs=[kv_full[qtr * NCORES * qs:(qtr + 1) * NCORES * qs, :]],
                        )

            with nc.named_scope("qproj"):
                for ib in range(NBLK):
                    sl = slice(ib * 128, (ib + 1) * 128)
                    ps = kvp.tile([128, 2 * C], F32)
                    for g in range(4):
                        nc.tensor.matmul(ps[:, 0:C], lhsT=zT[:, g, sl],
                                         rhs=wq_sb[:, g, :], start=(g == 0), stop=False)
                    nc.tensor.matmul(ps[:, 0:C], lhsT=ones_k1[:], rhs=bq_sb[:],
                                     start=False, stop=True)
                    nc.scalar.copy(q_sb[:, ib, :], ps[:, 0:C])

        # ---------------- phase E: edges + per-block tails ---------------
        with ExitStack() as pctx:
            kvp2 = pctx.enter_context(tc.tile_pool(name="kvp", bufs=2))
            idxp = pctx.enter_context(tc.tile_pool(name="idxp", bufs=2))
            ohp_ = pctx.enter_context(tc.tile_pool(name="ohp", bufs=2))
            ohtp = pctx.enter_context(tc.tile_pool(name="ohtp", bufs=2))
            bp = pctx.enter_context(tc.tile_pool(name="bp", bufs=2))
            qep = pctx.enter_context(tc.tile_pool(name="qep", bufs=2, space="PSUM"))
            scp = pctx.enter_context(tc.tile_pool(name="scp", bufs=2))
            wtp = pctx.enter_context(tc.tile_pool(name="wtp", bufs=3))
            pop_ = pctx.enter_context(tc.tile_pool(name="pout", bufs=2, space="PSUM"))
            psp = pctx.enter_context(tc.tile_pool(name="pssum", bufs=1, space="PSUM"))
            trp2 = pctx.enter_context(tc.tile_pool(name="trp2", bufs=1, space="PSUM"))
            opp = pctx.enter_context(tc.tile_pool(name="opsum", bufs=1, space="PSUM"))
            hp = pctx.enter_context(tc.tile_pool(name="hpsum", bufs=1, space="PSUM"))
            finp = pctx.enter_context(tc.tile_pool(name="finp", bufs=2))
            lnp2 = pctx.enter_context(tc.tile_pool(name="lnp2", bufs=2))

            # block tail: normalize + Wo + residual + LN2 + MLP
            def block_tail(rb, pout, pssum):
                sl = slice(rb * 128, (rb + 1) * 128)
                sm = finp.tile([128, H], F32, tag="sm")
                nc.vector.tensor_scalar(sm[:], pssum[:], 1e-30, None, op0=ALU.max)
                rec = finp.tile([128, H], F32, tag="rec")
                nc.vector.reciprocal(rec[:], sm[:])
                att = finp.tile([128, C], BF16, tag="att")
                nc.vector.tensor_tensor(_phd(att[:]), _phd(pout[:]),
                                        _bc(rec[:], D), op=ALU.mult)
                attT = finp.tile([128, 4, 128], BF16, tag="attT")
                for g in range(4):
                    pt = trp2.tile([128, 128], BF16)
                    nc.tensor.transpose(pt[:], att[:, g * 128:(g + 1) * 128], ident[:])
                    nc.scalar.copy(attT[:, g, :], pt[:])
                po = opp.tile([128, C], F32, tag="acc")
                for g in range(4):
                    nc.tensor.matmul(po[:], lhsT=attT[:, g, :], rhs=wo_sb[:, g, :],
                                     start=(g == 0), stop=False)
                nc.tensor.matmul(po[:], lhsT=ones_k1[:], rhs=bo_sb[:],
                                 start=False, stop=True)
                xb2 = finp.tile([128, C], F32, tag="xb2")
                nc.sync.dma_start(out=xb2[:], in_=x_c[sl, :])
                x1t = finp.tile([128, C], F32, tag="x1t")
                nc.vector.tensor_add(x1t[:], po[:], xb2[:])
                z2 = layernorm(finp, lnp2, x1t)
                z2T = finp.tile([128, 4, 128], BF16, tag="z2T")
                for g in range(4):
                    pt = trp2.tile([128, 128], BF16)
                    nc.tensor.transpose(pt[:], z2[:, g * 128:(g + 1) * 128], ident[:])
                    nc.scalar.copy(z2T[:, g, :], pt[:])
                py = opp.tile([128, C], F32, tag="acc")
                for half in range(2):
                    ph = hp.tile([128, 4, 128], F32)
                    for j in range(4):
                        chc = half * 4 + j
                        csl = slice(chc * 128, (chc + 1) * 128)
                        for g in range(4):
                            nc.tensor.matmul(ph[:, j, :], lhsT=w1_sb[:, g, csl],
                                             rhs=z2T[:, g, :], start=(g == 0), stop=False)
                        nc.tensor.matmul(ph[:, j, :], lhsT=b1_sb[:, csl],
                                         rhs=ones_k1[:], start=False, stop=True)
                    hs = finp.tile([128, 4, 128], BF16, tag="hs")
                    nc.scalar.activation(hs[:], ph[:], AF.Silu)
                    for j in range(4):
                        chc = half * 4 + j
                        nc.tensor.matmul(py[:], lhsT=hs[:, j, :], rhs=w2_sb[:, chc, :],
                                         start=(chc == 0), stop=False)
                nc.tensor.matmul(py[:], lhsT=ones_k1[:], rhs=b2_sb[:],
                                 start=False, stop=True)
                yt = finp.tile([128, C], F32, tag="yt")
                nc.vector.tensor_add(yt[:], py[:], x1t[:])
                nc.sync.dma_start(out=y_out[sl, :], in_=yt[:])

            pout = pssum = None
            wt_count = 0.0
            for ch in range(NCH):
                t0 = ch * CHUNK_T
                tiles_c = min(CHUNK_T, NT - t0)
                n_idx = tiles_c * 128
                cidx = idxp.tile([128, CHUNK_T * 8], I16, tag="cidx")
                nc.sync.dma_start(out=cidx[:], in_=colw[ch * 128:(ch + 1) * 128, :])
                kvb = kvp2.tile([128, CHUNK_T, EB], FP8)
                nc.gpsimd.dma_gather(
                    out_ap=kvb[:, :tiles_c, :], in_ap=kv_full[:],
                    idxs_ap=cidx[:, :n_idx // 16],
                    num_idxs=n_idx, num_idxs_reg=n_idx, elem_size=EB,
                    single_packet=False)
                sl_t = slice(t0 * 128, (t0 + tiles_c) * 128)
                ohc = ohp_.tile([128, CHUNK_T * 128], FP8, tag="ohc")
                nc.sync.dma_start(out=ohc[:, :n_idx], in_=ohP_d[:, sl_t])
                ohtc = ohtp.tile([128, CHUNK_T * 128], FP8, tag="ohtc")
                nc.sync.dma_start(out=ohtc[:, :n_idx], in_=ohT_d[:, sl_t])
                biac = bp.tile([128, CHUNK_T, H], F32, tag="bia")
                nc.sync.dma_start(
                    out=biac[:, :tiles_c, :],
                    in_=biasT_d[:, t0 * H:(t0 + tiles_c) * H])

                # pass 1: Qe matmul + scan -> per-head totals
                ends = scp.tile([128, CHUNK_T, H + 1], F32, tag="ends")
                nc.vector.memset(ends[:], 0.0)
                for s in range(tiles_c):
                    esl = slice(s * 128, (s + 1) * 128)
                    qe = qep.tile([128, C], F32)
                    nc.tensor.matmul(qe[:], lhsT=ohtc[:, esl], rhs=q_sb[:, (t0 + s) // T_BLK, :],
                                     start=True, stop=True)
                    out_ap = _bc(ends[:, s, 1:H + 1], D)  # [128, H, 64] step-0 inner
                    nc.vector._custom_dve(
                        scan_op, out=out_ap,
                        in0=_phd(kvb[:, s, 0:KB]), in1=_phd(qe[:]))
                # pass 2 (per chunk): ends are cumulative within each tile ->
                # per-head scores by differencing, + bias, + batched exp.
                sc2 = scp.tile([128, CHUNK_T, H], F32, tag="sc2")
                nc.vector.tensor_tensor(sc2[:, :tiles_c, :], ends[:, :tiles_c, 1:H + 1],
                                        ends[:, :tiles_c, 0:H], op=ALU.subtract)
                sc3 = scp.tile([128, CHUNK_T, H], F32, tag="sc3")
                nc.vector.tensor_tensor(sc3[:, :tiles_c, :], sc2[:, :tiles_c, :],
                                        biac[:, :tiles_c, :], op=ALU.add)
                pc = scp.tile([128, CHUNK_T, H], BF16, tag="pc")
                nc.scalar.activation(pc[:, :tiles_c, :], sc3[:, :tiles_c, :], AF.Exp)

                # pass 3: wt + scatter (+ block tails)
                for s in range(tiles_c):
                    t = t0 + s
                    esl = slice(s * 128, (s + 1) * 128)
                    rb, tb = divmod(t, T_BLK)
                    if tb == 0:
                        pout = pop_.tile([128, C], F32)
                        pssum = psp.tile([128, H], F32)
                    wt = wtp.tile([128, C], BF16, tag="wt")
                    wt_count += WT_SPLIT
                    eng = nc.gpsimd if wt_count >= 1.0 else nc.vector
                    if wt_count >= 1.0:
                        wt_count -= 1.0
                    eng.tensor_tensor(_phd(wt[:]), _phd(kvb[:, s, KB:EB]),
                                      _bc(pc[:, s, :], D), op=ALU.mult)
                    nc.tensor.matmul(pout[:], lhsT=ohc[:, esl], rhs=wt[:],
                                     start=(tb == 0), stop=(tb == T_BLK - 1))
                    nc.tensor.matmul(pssum[:], lhsT=ohc[:, esl],
                                     rhs=pc[:, s, :].rearrange("p h -> p h ()"),
                                     start=(tb == 0), stop=(tb == T_BLK - 1))
                    if tb == T_BLK - 1:
                        block_tail(rb, pout, pssum)

    nc.finalize()
    _split_multi_waits(nc)
    return nc


# --------------------------------------------------------------------------
# entry point
# --------------------------------------------------------------------------

def kernel(**inputs) -> np.ndarray:
    x = np.asarray(inputs["x"], np.float32)
    row = np.asarray(inputs["row_index"]).astype(np.int64)
    col = np.asarray(inputs["col_index"]).astype(np.int64)
    att_bias = np.asarray(inputs["att_bias"], np.float32)
    L = x.shape[0]
    LSH = L // NCORES

    T_BLK, NT, NCH, cores = _preprocess_edges(L, row, col, att_bias)
    w = _prep_weights(inputs)

    key = (L, T_BLK, NT, NCH)
    if key not in _prog_cache:
        _prog_cache[key] = _build_program(L, T_BLK, NT, NCH)
    nc = _prog_cache[key]

    in_maps = []
    for c in range(NCORES):
        m = dict(w)
        m["x_c"] = np.ascontiguousarray(x[c * LSH:(c + 1) * LSH])
        m.update(cores[c])
        in_maps.append(m)

    global LAST_EXEC_NS, LAST_RESULTS
    res = run_bass_kernel_spmd(nc, in_maps, list(range(NCORES)), trace=TRACE)
    LAST_RESULTS = res
    LAST_EXEC_NS = res.exec_time_ns
    return np.concatenate([res.results[c]["y"] for c in range(NCORES)], axis=0)
